# revision 31
# baseline (speedup 1.0000x reference)
"""MedianPool2d (3x3, stride 1, zero-pad 1) Trainium2 Bass kernel.

Full input x: (8, 64, 256, 256) fp32.  Sharding: pure data parallel over
batch -> core i processes x[i] (64, 256, 256).

The kernel computes in bfloat16: the median is a pure selection network
(min/max only), and rounding to bf16 is monotone, so the result is
exactly bf16(median_fp32(x)) -- max rel err ~0.4%, far inside the 2e-2
gate -- while DVE tensor_tensor runs at 2 elem/cycle/lane instead of 1.

Layout (host-prepared, zero compute on device for padding/casting):
  Per core the input is rearranged to xp[c, 258, 258] bf16:
    row t = global row t-1 (rows 0 and 257 are the zero halo);
    within a row, columns are parity-split with pads baked in:
      idx 0       = 0 (left halo, plays B[-1])
      idx 1+k     = col 2k+1 (odd cols),  k = 0..127   ("B")
      idx 129+k   = col 2k   (even cols), k = 0..127   ("A")
      idx 257     = 0 (right halo, plays A[128])
  The 3-tap window of output col c maps to unit-stride slices:
    even c=2k:  {B[k-1], A[k], B[k]} = {V[0:128], V[129:257], V[1:129]}
    odd  c=2k+1:{A[k], B[k], A[k+1]} = {V[129:257], V[1:129], V[130:258]}
  so every tensor_tensor AP has innermost step 1 (keeps the 2x DVE mode;
  stride-2 APs would drop to 1x).

SBUF partitions: p = h*64 + ch, h in {0,1} = top/bottom 128-row half.
Per chunk of R output rows: vertical sliding sort3 with row-pair sharing
(5 ops/px), horizontal merge with even/odd pair sharing (6 ops/px),
final med3 (4 ops/px).  Even/odd completions are fused into single
instructions via zero-stride broadcast APs.  Output is written
parity-split bf16 ([0:128]=even cols, [128:256]=odd) and re-interleaved
+ upcast on the host.
"""

import numpy as np

B, C, H, W = 8, 64, 256, 256
NCORES = 8
HH = H // 2          # rows per half-strip
WL = 258             # parity-split padded row width
HP = H + 2           # padded row count

_CACHE = {}


def _bcast_parity(ap3, last=None):
    """[p, r, w] AP -> [p, r, (0,2), w]: broadcast over the parity dim."""
    import concourse.mybir as mybir
    from concourse.ap import AP

    dims = [list(d) for d in ap3.ap]
    if last is not None:
        dims[-1][1] = last
    new = dims[:-1] + [[0, 2]] + [dims[-1]]
    return AP(tensor=ap3.tensor, offset=ap3.offset,
              ap=mybir.VecI64Pair(new))


def _thirds(ap3, pstride):
    """[p, r, w] AP -> [p, r, (pstride,2), (1,128)]: the two parity
    third-element slices (j=0: cols 0..127, j=1: cols pstride..)."""
    import concourse.mybir as mybir
    from concourse.ap import AP

    dims = [list(d) for d in ap3.ap]
    dims[-1] = [1, 128]
    new = dims[:-1] + [[pstride, 2]] + [dims[-1]]
    return AP(tensor=ap3.tensor, offset=ap3.offset,
              ap=mybir.VecI64Pair(new))


def _build_bf16(R=32, in_bufs=2, out_bufs=2, taper=0, in_fused=True,
                out_fused=False):
    """Bass module for one core: xp (64,258,258) bf16 -> out (64,256,256)
    bf16 parity-split.  taper>0 splits the first/last chunk into
    taper-row pieces so the pipeline fills/drains faster."""
    import concourse.bacc as bacc
    import concourse.mybir as mybir
    from concourse.tile import TileContext

    MIN = mybir.AluOpType.min
    MAX = mybir.AluOpType.max
    bf16 = mybir.dt.bfloat16

    if isinstance(R, (tuple, list)):
        chunks = list(R)
    elif taper:
        assert taper % 2 == 0 and R % taper == 0
        chunks = ([taper] * (R // taper) + [R] * (HH // R - 2)
                  + [taper] * (R // taper))
    else:
        chunks = [R] * (HH // R)
    assert sum(chunks) == HH and all(r % 2 == 0 for r in chunks)

    nc = bacc.Bacc("TRN2", name="median_pool2d_bf16")
    x = nc.dram_tensor("x", [C, HP, WL], bf16, kind="ExternalInput")
    out = nc.dram_tensor("out", [C, H, W], bf16, kind="ExternalOutput")
    xg = x.ap()
    og = out.ap()

    with TileContext(nc) as tc:
        with (
            tc.tile_pool(name="io_in", bufs=in_bufs) as in_pool,
            tc.tile_pool(name="io_out", bufs=out_bufs) as out_pool,
            tc.tile_pool(name="work", bufs=1) as w_pool,
        ):
            def wtile(name, rows, width, tag=None):
                t = w_pool.tile([128, rows * width], bf16, name=name,
                                tag=tag or name)
                return t.rearrange("p (r w) -> p r w", w=width)

            from concourse.ap import AP as APc

            r0 = 0
            for ci, R in enumerate(chunks):
                Rh = R // 2
                it = in_pool.tile([128, (R + 2) * WL], bf16, name="it",
                                  tag="it")
                it3 = it.rearrange("p (r w) -> p r w", w=WL)
                # One fused DMA covers both image halves: partition p=h*64+c
                # maps to DRAM rows (r0 + h*HH ..) of channel c (the DGE and
                # the DMA bus are single serialized resources, so separate
                # half-DMAs buy nothing).  The first chunk is split by
                # column so the vertical stage starts after half a transfer.
                S_it = (R + 2) * WL
                if in_fused:
                    nc.sync.dma_start(
                        out=it[:],
                        in_=APc(tensor=xg.tensor, offset=r0 * WL,
                                ap=mybir.VecI64Pair(
                                    [[HH * WL, 2], [HP * WL, 64],
                                     [1, S_it]])))
                else:
                    nc.sync.dma_start(out=it3[0:64],
                                      in_=xg[:, r0:r0 + R + 2, :])
                    nc.scalar.dma_start(
                        out=it3[64:128],
                        in_=xg[:, HH + r0:HH + r0 + R + 2, :])

                # ---- vertical sliding sort3 (rows); all 258 cols are
                # consumed by the merge ----
                WV = WL
                Pm = wtile("Pm", Rh, WL)
                PM = wtile("PM", Rh, WL)
                Lo = wtile("Lo", R, WL)
                Hi = wtile("Hi", R, WL)
                tQ = wtile("tQ", R, WL, tag="mB")
                Me = wtile("Me", R, WL)
                tdims = [list(d) for d in it3[:].ap]  # [p,(260,R+2),(1,260)]
                vsplit = (0, WV)
                for c0, c1 in zip(vsplit, vsplit[1:]):
                    nc.vector.tensor_tensor(out=Pm[:, :, c0:c1],
                                            in0=it3[:, 1:R + 1:2, c0:c1],
                                            in1=it3[:, 2:R + 2:2, c0:c1],
                                            op=MIN)
                    nc.vector.tensor_tensor(out=PM[:, :, c0:c1],
                                            in0=it3[:, 1:R + 1:2, c0:c1],
                                            in1=it3[:, 2:R + 2:2, c0:c1],
                                            op=MAX)
                    # thirds: out row 2i -> it3 row 2i; odd rows -> 2i+3
                    thr = type(it3[:])(
                        tensor=it3[:].tensor, offset=it3[:].offset + c0,
                        ap=mybir.VecI64Pair(
                            [tdims[0], [2 * WL, Rh], [3 * WL, 2],
                             [1, c1 - c0]]))
                    Pm_b = _bcast_parity(Pm[:, :, c0:c1])
                    PM_b = _bcast_parity(PM[:, :, c0:c1])
                    nc.vector.tensor_tensor(out=Lo[:, :, c0:c1], in0=thr,
                                            in1=Pm_b, op=MIN)
                    nc.vector.tensor_tensor(out=Hi[:, :, c0:c1], in0=thr,
                                            in1=PM_b, op=MAX)
                    nc.vector.tensor_tensor(out=tQ[:, :, c0:c1], in0=thr,
                                            in1=PM_b, op=MIN)
                    nc.vector.tensor_tensor(out=Me[:, :, c0:c1], in0=Pm_b,
                                            in1=tQ[:, :, c0:c1], op=MAX)

                # ---- horizontal merge: A=max3(Lo), C=min3(Hi), B=med3(Me)
                def halves(V):
                    return V[:, :, 129:257], V[:, :, 1:129]

                PA = wtile("PA", R, 128, tag="tQ")
                mA = wtile("mA", R, 256)
                a1, b1 = halves(Lo)
                nc.vector.tensor_tensor(out=PA[:], in0=a1, in1=b1, op=MAX)
                nc.vector.tensor_tensor(out=mA[:], in0=_bcast_parity(PA[:]),
                                        in1=_thirds(Lo[:], 130), op=MAX)

                PC = wtile("PC", R, 128, tag="tQ")
                mC = wtile("mC", R, 256)
                a2, b2 = halves(Hi)
                nc.vector.tensor_tensor(out=PC[:], in0=a2, in1=b2, op=MIN)
                nc.vector.tensor_tensor(out=mC[:], in0=_bcast_parity(PC[:]),
                                        in1=_thirds(Hi[:], 130), op=MIN)

                Um = wtile("Um", R, 128, tag="Pm")
                Vm = wtile("Vm", R, 128, tag="PM")
                a3, b3 = halves(Me)
                nc.vector.tensor_tensor(out=Um[:], in0=a3, in1=b3, op=MIN)
                nc.vector.tensor_tensor(out=Vm[:], in0=a3, in1=b3, op=MAX)
                tB = wtile("tB", R, 256, tag="Lo")
                mB = wtile("mB", R, 256)
                nc.vector.tensor_tensor(out=tB[:], in0=_bcast_parity(Vm[:]),
                                        in1=_thirds(Me[:], 130), op=MIN)
                nc.vector.tensor_tensor(out=mB[:], in0=_bcast_parity(Um[:]),
                                        in1=tB[:], op=MAX)

                # ---- final med3(A, B, C) ----
                mT = wtile("mT", R, 256, tag="Hi")
                mU = wtile("mU", R, 256, tag="Me")
                mV = wtile("mV", R, 256, tag="Lo")
                ot = out_pool.tile([128, R * 256], bf16, name="ot", tag="ot")
                ot3 = ot.rearrange("p (r w) -> p r w", w=256)
                nc.vector.tensor_tensor(out=mT[:], in0=mA[:], in1=mB[:],
                                        op=MIN)
                nc.vector.tensor_tensor(out=mU[:], in0=mA[:], in1=mB[:],
                                        op=MAX)
                nc.vector.tensor_tensor(out=mV[:], in0=mU[:], in1=mC[:],
                                        op=MIN)
                # Last chunk: split the final op + store by column half so
                # the first half's out-DMA overlaps the second half's
                # compute, shrinking the drain tail.
                nc.vector.tensor_tensor(out=ot3[:], in0=mT[:], in1=mV[:],
                                        op=MAX)
                if out_fused or ci == len(chunks) - 1:
                    nc.scalar.dma_start(
                        out=APc(tensor=og.tensor, offset=r0 * W,
                                ap=mybir.VecI64Pair(
                                    [[HH * W, 2], [H * W, 64],
                                     [1, R * W]])),
                        in_=ot[:])
                else:
                    nc.sync.dma_start(out=og[:, r0:r0 + R, :],
                                      in_=ot3[0:64])
                    nc.sync.dma_start(out=og[:, HH + r0:HH + r0 + R, :],
                                      in_=ot3[64:128])
                r0 += R

    nc.compile()
    return nc


def _build_bf16_v2(chunks=(16, 28, 28, 28, 28), in_bufs=2, out_bufs=2):
    """Merged-instruction variant: 12 DVE instructions per chunk.

    All work buffers live in one mega tile so that pairs of same-ALU-op
    instructions can be fused into single stacked-AP instructions
    (2-element outer dim spanning both destination regions).  Regions
    (S = R*WL elems/partition):
      s0 Lo | s1 tQ->mA | s2 Me->mB | s3 Hi->mT | s4 tB | s5 mC
      s6 mU | s7 mV | s8..s10: Pm/PM then PA/Um/Vm/PC
    mA/mB/mC/tB/mT/mU/mV are stored 131-parity (even half at col 0, odd
    half at col 131 of each WL-wide row).
    """
    import concourse.bacc as bacc
    import concourse.mybir as mybir
    from concourse.ap import AP
    from concourse.tile import TileContext

    MIN = mybir.AluOpType.min
    MAX = mybir.AluOpType.max
    bf16 = mybir.dt.bfloat16

    chunks = list(chunks)
    assert sum(chunks) == HH and all(r % 2 == 0 for r in chunks)

    nc = bacc.Bacc("TRN2", name="median_pool2d_bf16v2")
    x = nc.dram_tensor("x", [C, HP, WL], bf16, kind="ExternalInput")
    out = nc.dram_tensor("out", [C, H, W], bf16, kind="ExternalOutput")
    xg = x.ap()
    og = out.ap()
    WV = WL - 1

    with TileContext(nc) as tc:
        with (
            tc.tile_pool(name="io_in", bufs=in_bufs) as in_pool,
            tc.tile_pool(name="io_out", bufs=out_bufs) as out_pool,
            tc.tile_pool(name="work", bufs=1) as w_pool,
        ):
            r0 = 0
            for R in chunks:
                Rh = R // 2
                S = R * WL
                hp = R * 128
                mg = w_pool.tile([128, 10 * S], bf16, name="mega",
                                 tag="mega")
                mga = mg[:]
                pd = list(mga.ap[0])

                def rap(off, dims, t=None):
                    base = mga if t is None else t
                    return AP(tensor=base.tensor, offset=base.offset + off,
                              ap=mybir.VecI64Pair([list(base.ap[0])]
                                                  + [list(d) for d in dims]))

                it = in_pool.tile([128, (R + 2) * WL], bf16, name="it",
                                  tag="it")
                it3 = it.rearrange("p (r w) -> p r w", w=WL)
                nc.sync.dma_start(out=it3[0:64],
                                  in_=xg[:, r0:r0 + R + 2, :])
                nc.scalar.dma_start(out=it3[64:128],
                                    in_=xg[:, HH + r0:HH + r0 + R + 2, :])

                # 1-2: vertical pair sort -> Pm @8S, PM @8S+Rh*WL
                pm_o, pM_o = 8 * S, 8 * S + Rh * WL
                for off, op in ((pm_o, MIN), (pM_o, MAX)):
                    nc.vector.tensor_tensor(
                        out=rap(off, [[WL, Rh], [1, WV]]),
                        in0=it3[:, 1:R + 1:2, 0:WV],
                        in1=it3[:, 2:R + 2:2, 0:WV], op=op)

                # thirds into it3: out row 2i+j reads it3 row 2i+3j
                thr1 = rap(0, [[2 * WL, Rh], [3 * WL, 2], [1, WV]], t=it3[:])
                thr2 = rap(0, [[0, 2], [2 * WL, Rh], [3 * WL, 2], [1, WV]],
                           t=it3[:])

                # 3: {Lo@0, tQ@S} = min(thr, [Pm; PM])
                nc.vector.tensor_tensor(
                    out=rap(0, [[S, 2], [2 * WL, Rh], [WL, 2], [1, WV]]),
                    in0=thr2,
                    in1=rap(pm_o, [[Rh * WL, 2], [WL, Rh], [0, 2], [1, WV]]),
                    op=MIN)
                # 4: Hi@3S = max(thr, PM_b)
                nc.vector.tensor_tensor(
                    out=rap(3 * S, [[2 * WL, Rh], [WL, 2], [1, WV]]),
                    in0=thr1,
                    in1=rap(pM_o, [[WL, Rh], [0, 2], [1, WV]]), op=MAX)
                # 5: Me@2S = max(Pm_b, tQ)
                nc.vector.tensor_tensor(
                    out=rap(2 * S, [[2 * WL, Rh], [WL, 2], [1, WV]]),
                    in0=rap(pm_o, [[WL, Rh], [0, 2], [1, WV]]),
                    in1=rap(S, [[2 * WL, Rh], [WL, 2], [1, WV]]), op=MAX)

                # P4 region overwrites Pm/PM: PA@8S, Um@+h, Vm@+2h, PC@+3h
                pa_o, um_o, vm_o, pc_o = (8 * S, 8 * S + hp, 8 * S + 2 * hp,
                                          8 * S + 3 * hp)
                AB = lambda off: (rap(off + 130, [[WL, R], [1, 128]]),
                                  rap(off + 1, [[WL, R], [1, 128]]))
                # 6: {PA, Vm} = max([Lo-A; Me-A], [Lo-B; Me-B])
                nc.vector.tensor_tensor(
                    out=rap(pa_o, [[2 * hp, 2], [128, R], [1, 128]]),
                    in0=rap(130, [[2 * S, 2], [WL, R], [1, 128]]),
                    in1=rap(1, [[2 * S, 2], [WL, R], [1, 128]]), op=MAX)
                # 7: {Um, PC} = min([Me-A; Hi-A], [Me-B; Hi-B])
                nc.vector.tensor_tensor(
                    out=rap(um_o, [[2 * hp, 2], [128, R], [1, 128]]),
                    in0=rap(2 * S + 130, [[S, 2], [WL, R], [1, 128]]),
                    in1=rap(2 * S + 1, [[S, 2], [WL, R], [1, 128]]), op=MIN)

                def par(off, stack=None):
                    dims = [[WL, R], [131, 2], [1, 128]]
                    if stack is not None:
                        dims = [stack] + dims
                    return rap(off, dims)

                # 8: {tB@4S, mC@5S} = min([Vm_b; PC_b], [Me-th; Hi-th])
                nc.vector.tensor_tensor(
                    out=par(4 * S, [S, 2]),
                    in0=rap(vm_o, [[hp, 2], [128, R], [0, 2], [1, 128]]),
                    in1=par(2 * S, [S, 2]), op=MIN)
                # 9: {mA@S, mB@2S} = max([PA_b; Um_b], [Lo-th; tB-th])
                nc.vector.tensor_tensor(
                    out=par(S, [S, 2]),
                    in0=rap(pa_o, [[hp, 2], [128, R], [0, 2], [1, 128]]),
                    in1=par(0, [4 * S, 2]), op=MAX)
                # 10: mU@6S = max(mA, mB)
                nc.vector.tensor_tensor(out=par(6 * S), in0=par(S),
                                        in1=par(2 * S), op=MAX)
                # 11: {mT@3S, mV@7S} = min([mA; mU], [mB; mC])
                nc.vector.tensor_tensor(
                    out=par(3 * S, [4 * S, 2]),
                    in0=par(S, [5 * S, 2]),
                    in1=par(2 * S, [3 * S, 2]), op=MIN)

                # 12: ot = max(mT, mV) -> plain parity-256 layout
                ot = out_pool.tile([128, R * 256], bf16, name="ot", tag="ot")
                ot3 = ot.rearrange("p (r w) -> p r w", w=256)
                nc.vector.tensor_tensor(
                    out=rap(0, [[256, R], [128, 2], [1, 128]], t=ot3[:]),
                    in0=par(3 * S), in1=par(7 * S), op=MAX)

                nc.sync.dma_start(out=og[:, r0:r0 + R, :], in_=ot3[0:64])
                nc.scalar.dma_start(out=og[:, HH + r0:HH + r0 + R, :],
                                    in_=ot3[64:128])
                r0 += R

    nc.compile()
    return nc


def _get_nc(R=(4, 30, 30, 30, 30, 4), in_bufs=2, out_bufs=2, taper=0,
            v2=False):
    key = (tuple(R) if isinstance(R, (tuple, list)) else R,
           in_bufs, out_bufs, taper, v2)
    if key not in _CACHE:
        if v2:
            _CACHE[key] = _build_bf16_v2(chunks=R, in_bufs=in_bufs,
                                         out_bufs=out_bufs)
        else:
            _CACHE[key] = _build_bf16(R=R, in_bufs=in_bufs,
                                      out_bufs=out_bufs, taper=taper)
    return _CACHE[key]


def _prep_core(xi, bf16):
    """(64,256,256) f32 -> (64,258,260) bf16 parity-split padded."""
    xp = np.zeros((C, HP, WL), dtype=bf16)
    xb = xi.astype(bf16)
    xp[:, 1:H + 1, 1:129] = xb[:, :, 1::2]
    xp[:, 1:H + 1, 129:257] = xb[:, :, 0::2]
    return xp


def kernel(x: np.ndarray) -> np.ndarray:
    """MedianPool2d(3x3, s=1, p=1) on 8 NeuronCores, bf16 selection
    network (exact median of the bf16-rounded input)."""
    import ml_dtypes
    from concourse.bass_utils import run_bass_kernel_spmd

    bf16 = ml_dtypes.bfloat16
    assert x.shape == (B, C, H, W), x.shape
    x = np.ascontiguousarray(x, dtype=np.float32)
    nc = _get_nc()
    in_maps = [{"x": _prep_core(x[i], bf16)} for i in range(NCORES)]
    res = run_bass_kernel_spmd(nc, in_maps, core_ids=list(range(NCORES)))
    y = np.empty((B, C, H, W), dtype=np.float32)
    for i in range(NCORES):
        o = res.results[i]["out"]
        y[i, :, :, 0::2] = o[:, :, 0:128]
        y[i, :, :, 1::2] = o[:, :, 128:256]
    return y
